# revision 25
# baseline (speedup 1.0000x reference)
"""Causal MHA with RoPE on 8 trn2 NeuronCores.

Problem: x[2,2048,1024], 16 heads x 64, fp32, causal, RoPE, Wq/Wk/Wv/Wo.

Sharding: core c handles batch b = c//4 and head group g = c%4 (4 heads,
256 feature rows). Each core computes its partial output contribution
out_partial = attnout_g @ Wo[:, g_slice].T of shape [2048, 1024]; the host
sums the 4 partials per batch.

Software-pipelined schedule: the softmax exp stream (ScalarE-bound,
~82us total) overlaps the projection tail and the output projection
instead of running between them:

  front : x/wq/wk DMA-paced Q-proj, ropeQ, K-proj(m0), ropeK(m0)
  W1    : scores+exp h0   | fill: K-proj(m1), ropeK(m1), V g0
  W2    : scores+exp h1   | fill: V g1-g3, then PV h0 + div h0
  W3    : scores+exp h2   | PV h1, div h1
  W4    : scores+exp h3   | PV h2, div h2
  W5    : PV h3 + div h3  | out-proj interleaved by qt readiness

One PSUM pool for the whole kernel: tag "sc" = 2 rotating [128,1024]
slots (scores pairs / bc broadcasts / out-proj banks; also sliced into
4+4 Q/K-proj accumulators), tags pv0-3 = 4 [128,512] banks (m1-proj
accumulators, V-proj groups, then per-head PV accumulators).  The exp
outputs (pt) are buffered ~1 head deep in SBUF so PV can lag scores.

Per-core data layout (unchanged from the phase-serial version):
  xT   [1024, 2048] = x[b].T ; wqT/wkT/wvT [1024, 256]; woT [256, 1024]
  QT/KT [128, 512] tiles (two heads per QT tile; KT zero-padded per head)
  V    [128, 65]x16 k-chunks ([seq-chunk, head_dim+ones-col]; the ones
       column makes the PV matmul also produce softmax denominators)
  scores computed transposed (S^T[k, q] = K @ Q^T) so softmax sums run
  over the PSUM partition axis via the V ones-column; causal masks are
  [128,128] 0/1 multiplies on diagonal blocks only (GPSIMD).
"""

import numpy as np

B, S, D, H, HD = 2, 2048, 1024, 16, 64
NCORES = 8
GH = 4  # heads per core
GD = GH * HD  # 256
P = 128
NDC = D // P  # 8 feature chunks
NST = S // P  # 16 seq chunks of 128
QW = 512  # q tile width
NQT = S // QW  # 4
NKC = S // P  # 16 k chunks
SCALE = float(HD) ** -0.5

MM_MODE = "bf16"
NWARM = 110  # dummy PE matmuls bridging the input-DMA window
NPT = 28  # exp-output (pt) ring depth: ~1 head of scores stays buffered
NKA = 6  # HAM-keepalive dummy matmuls per ACT-bound iteration

_cache = {}


def _install_shims():
    """Make TileContext kernels compile+profile in this environment."""
    import sys
    import types

    if "antenv.axon_hooks" not in sys.modules:
        mod = types.ModuleType("antenv.axon_hooks")
        mod._hook = None

        def set_axon_ntff_profile_hook(h):
            mod._hook = h

        def get_axon_ntff_profile_hook():
            return mod._hook

        mod.set_axon_ntff_profile_hook = set_axon_ntff_profile_hook
        mod.get_axon_ntff_profile_hook = get_axon_ntff_profile_hook
        sys.modules["antenv.axon_hooks"] = mod
        import antenv

        antenv.axon_hooks = mod
        try:
            from trn_agent_boot.trn_boot import _ntff_profile_via_ctypes

            hook = _ntff_profile_via_ctypes("/opt/axon/libaxon_pjrt.so")
            if hook is not None:
                mod.set_axon_ntff_profile_hook(hook)
        except Exception:
            pass
        try:
            import concourse.bass_utils as bu

            bu.upload_artifacts = lambda tmpdir: f"file://{tmpdir}"
        except Exception:
            pass

    import concourse.tile as tile_mod
    import concourse.mybir as mybir
    from concourse.vector_clock import ScopedClock

    if getattr(tile_mod.TileContext, "_tail_drain_patched", False):
        return

    def _drain_and_barrier(self, tick_clock, wait_clock):
        # The image's walrus rejects >1 sync wait per SP CTRL instruction;
        # spread the kernel-tail waits over single-wait NOPs.
        nc = self.nc
        nop = nc.sync.nop(nofuse=True)
        wait_clock.add_sem_waits(nop.ins, ScopedClock({None: tick_clock.global_clock}))
        si = nop.ins.sync_info
        if si is not None and si.on_wait and len(si.on_wait) > 1:
            extra = list(si.on_wait[1:])
            del si.on_wait[1:]
            for w in extra:
                n2 = nc.sync.nop(nofuse=True)
                si2 = n2.ins.sync_info
                if si2 is None:
                    n2.ins.sync_info = mybir.SyncInfo(on_wait=[w], on_update=[])
                else:
                    si2.on_wait.append(w)
        nc.sync.drain()
        nc.all_engine_barrier()
        assert self.sems is not None
        popped = nc._tile_sem_poison_stack.pop()
        assert popped is self._sem_poison
        nc.clear_and_free_semaphores(list(self.sems.allocated().values()))
        nc.all_engine_barrier()

    tile_mod.TileContext._drain_and_barrier = _drain_and_barrier
    tile_mod.TileContext._tail_drain_patched = True


MAX_WAITS = 1  # walrus in this image allows only 1 sync wait per instruction


def _split_excess_waits(nc, max_waits=MAX_WAITS):
    """Spill excess per-instruction sem waits onto same-engine NOPs."""
    import concourse.mybir as mybir

    n = 0
    for f in nc.m.functions:
        for bb in f.blocks:
            new_insts = []
            for inst in bb.instructions:
                si = inst.sync_info
                if si is not None and si.on_wait and len(si.on_wait) > max_waits:
                    extra = list(si.on_wait[: -max_waits])
                    keep = list(si.on_wait[-max_waits:])
                    for i in range(0, len(extra), max_waits):
                        chunk = extra[i : i + max_waits]
                        n += 1
                        nop = mybir.InstNoOp(
                            name=f"waitsplit-{n}",
                            ins=[],
                            outs=[],
                            engine=inst.engine,
                            sync_info=mybir.SyncInfo(on_wait=chunk, on_update=[]),
                        )
                        new_insts.append(nop)
                    del si.on_wait[:]
                    si.on_wait.extend(keep)
                new_insts.append(inst)
            bb.instructions[:] = new_insts
    return n


def build_nc(split_waits=True):
    """Build the per-core Bass program (SPMD: same program on all 8 cores)."""
    key = ("nc", split_waits)
    if key in _cache:
        return _cache[key]
    _install_shims()

    import concourse.bass as bass
    import concourse.mybir as mybir
    import concourse.tile as tile

    dt = mybir.dt
    f32 = dt.float32
    mdt = {"f32r": dt.float32r, "f32": dt.float32, "bf16": dt.bfloat16}[MM_MODE]

    Exp = mybir.ActivationFunctionType.Exp
    Ln = mybir.ActivationFunctionType.Ln
    AF_Copy = mybir.ActivationFunctionType.Copy

    from concourse import library_config  # noqa: F401  (import side effects)

    nc = bass.Bass()
    xT = nc.dram_tensor("xT", [D, S], mdt, kind="ExternalInput")
    wqT = nc.dram_tensor("wqT", [D, GD], mdt, kind="ExternalInput")
    wk0T = nc.dram_tensor("wk0T", [D, P], mdt, kind="ExternalInput")
    wk1T = nc.dram_tensor("wk1T", [D, P], mdt, kind="ExternalInput")
    wvT = nc.dram_tensor("wvT", [D, GD], mdt, kind="ExternalInput")
    woT = nc.dram_tensor("woT", [GD, D], mdt, kind="ExternalInput")
    cos2 = nc.dram_tensor("cos2", [P, S], mdt, kind="ExternalInput")
    sine = nc.dram_tensor("sine", [P, S], mdt, kind="ExternalInput")
    dmask = nc.dram_tensor("dmask", [P, P], mdt, kind="ExternalInput")
    rmatT = nc.dram_tensor("rmatT", [P, P], mdt, kind="ExternalInput")
    out = nc.dram_tensor("out", [S, D], mdt, kind="ExternalOutput")

    with tile.TileContext(nc) as tc:
        with (
            tc.tile_pool(name="sb", bufs=1) as pool,
            tc.tile_pool(name="ps", bufs=1, space="PSUM") as ps,
        ):
            # ---- persistent SBUF tiles ----
            cos_sb = pool.tile([P, S], mdt, tag="cos", name="cos_sb")
            sin_sb = pool.tile([P, S], mdt, tag="sin", name="sin_sb")
            dm_sb = pool.tile([P, P], mdt, tag="dm", name="dm_sb")
            ones1 = pool.tile([1, HD], mdt, tag="ones", name="ones1")
            rm_sb = pool.tile([P, P], mdt, tag="rm", name="rm_sb")
            warm_w = pool.tile([P, P], mdt, tag="warmw", name="warm_w")
            wo_sb = [
                pool.tile([P, D], mdt, tag=f"wo{c}", name=f"wo{c}") for c in range(2)
            ]
            QT = [
                [
                    pool.tile([P, QW], mdt, tag=f"qt{th}_{qt}", name=f"qt{th}_{qt}")
                    for qt in range(NQT)
                ]
                for th in range(2)
            ]
            KT = [
                [
                    pool.tile([P, QW], mdt, tag=f"kt{h}_{qt}", name=f"kt{h}_{qt}")
                    for qt in range(NQT)
                ]
                for h in range(GH)
            ]
            VA = [
                pool.tile([P, GH * (HD + 1)], mdt, tag=f"va{kc}", name=f"va{kc}")
                for kc in range(NKC)
            ]
            attnT = [
                [
                    pool.tile([P, QW], mdt, tag=f"at{th}_{qt}", name=f"at{th}_{qt}")
                    for qt in range(NQT)
                ]
                for th in range(2)
            ]
            x_sb = [
                pool.tile([P, S], mdt, tag=f"x{d_}", name=f"x{d_}") for d_ in range(NDC)
            ]
            w_sb = {}
            for wname in ("q", "v"):
                for d_ in range(NDC):
                    w_sb[wname, d_] = pool.tile(
                        [P, GD], mdt, tag=f"w{wname}{d_}", name=f"w{wname}{d_}"
                    )
            wk_sb = {}
            for m in range(2):
                for d_ in range(NDC):
                    wk_sb[m, d_] = pool.tile(
                        [P, P], mdt, tag=f"wk{m}{d_}", name=f"wk{m}{d_}"
                    )

            # ---- DMA emission ----
            # gpsimd (SWDGE) queue: constants by first use + SBUF memsets
            nc.gpsimd.memset(warm_w[:], 0.0)
            nc.gpsimd.dma_start(rm_sb[:], rmatT[:])
            nc.gpsimd.dma_start(cos_sb[:], cos2[:])
            nc.gpsimd.dma_start(sin_sb[:], sine[:])
            nc.gpsimd.dma_start(dm_sb[:], dmask[:])
            nc.gpsimd.memset(ones1[:], 1.0)
            for h in range(GH):
                zo = (1 - h % 2) * HD  # zero the other head's half
                for qt in range(NQT):
                    nc.gpsimd.memset(KT[h][qt][zo : zo + HD, :], 0.0)
            for d_ in range(NDC):
                nc.gpsimd.dma_start(w_sb["v", d_][:], wvT[d_ * P : (d_ + 1) * P, :])
            for kc in range(NKC):
                nc.gpsimd.memset(
                    VA[kc][:].rearrange("p (h c) -> p h c", c=HD + 1)[
                        :, :, HD : HD + 1
                    ],
                    1.0,
                )
            for c in range(2):
                nc.gpsimd.dma_start(wo_sb[c][:], woT[c * P : (c + 1) * P, :])
            # sync (HWDGE) queue: x/wq/wk0 interleaved per d-chunk so both
            # Q-proj and K-proj(m0) can accumulate as the stream lands; the
            # m1 half of wk follows (first needed mid-W1)
            for d_ in range(NDC):
                nc.sync.dma_start(x_sb[d_][:], xT[d_ * P : (d_ + 1) * P, :])
                nc.sync.dma_start(w_sb["q", d_][:], wqT[d_ * P : (d_ + 1) * P, :])
                nc.sync.dma_start(wk_sb[0, d_][:], wk0T[d_ * P : (d_ + 1) * P, :])
            for d_ in range(NDC):
                nc.sync.dma_start(wk_sb[1, d_][:], wk1T[d_ * P : (d_ + 1) * P, :])

            # ---- PSUM helpers ----
            def sc_tile(name):
                return ps.tile([P, 2 * QW], f32, tag="sc", bufs=2, name=name)

            def pv_tile(j, name):
                return ps.tile([P, QW], f32, tag=f"pv{j}", name=name)

            def keepalive(pvb, n=NKA):
                # Zero-result dummy matmuls into partitions 64-127 of a
                # PE-exclusive pv bank: hold the HAM activity monitor at full
                # clock through ACT-bound stretches.  warm_w is zeros and
                # start=False (accumulate where written / overwrite junk
                # elsewhere), so live rows only ever receive "+= 0".
                for _ in range(n):
                    nc.tensor.matmul(
                        pvb[64:128, 0:QW],
                        lhsT=warm_w[:, 0:64],
                        rhs=x_sb[0][:, 0:QW],
                        start=False,
                        stop=True,
                        skip_group_check=True,
                    )


            # ---- PE warmup over the input-DMA window ----
            warm_ps = sc_tile("warm")
            for _ in range(NWARM):
                nc.tensor.matmul(
                    warm_ps[:, 0:P], lhsT=warm_w[:], rhs=warm_w[:],
                    start=True, stop=True,
                )

            # ---- front projections: Q m0 (2 sc slots) and K m0 (4 pv
            # banks) accumulate concurrently, paced by the x/wq/wk0 DMA
            # stream; the m1 halves run inside W1 as PE filler ----
            qA, qB = sc_tile("qA"), sc_tile("qB")
            psq0 = [qA[:, 0:QW], qA[:, QW:], qB[:, 0:QW], qB[:, QW:]]
            psk0 = [pv_tile(j, f"psk0{j}") for j in range(NQT)]

            def qproj_group(m, d_, ps_aps):
                for st in range(NQT):
                    nc.tensor.matmul(
                        ps_aps[st],
                        lhsT=w_sb["q", d_][:, m * P : (m + 1) * P],
                        rhs=x_sb[d_][:, st * QW : (st + 1) * QW],
                        start=(d_ == 0),
                        stop=(d_ == NDC - 1),
                    )

            def km0_group(d_):
                for st in range(NQT):
                    nc.tensor.matmul(
                        psk0[st][:],
                        lhsT=wk_sb[0, d_][:],
                        rhs=x_sb[d_][:, st * QW : (st + 1) * QW],
                        start=(d_ == 0),
                        stop=(d_ == NDC - 1),
                    )

            for d_ in range(NDC):
                qproj_group(0, d_, psq0)
                km0_group(d_)
                if d_ > 0:
                    keepalive(psk0[3][:], 2)

            def rope_group(wname, dst, m, ps_aps):
                for st in range(NQT):
                    p_ = ps_aps[st]
                    sl = slice(st * QW, (st + 1) * QW)
                    raw = pool.tile(
                        [P, QW], mdt, tag="raw", bufs=3, name=f"raw{wname}{m}{st}"
                    )
                    # PSUM->SBUF drain on ACT (idle through the proj phase)
                    nc.scalar.copy(raw[:], p_)
                    # rotate_half on the PE: p_ <- R @ raw (in place)
                    nc.tensor.matmul(
                        p_, lhsT=rm_sb[:], rhs=raw[:], start=True, stop=True
                    )
                    m1 = pool.tile(
                        [P, QW], mdt, tag="m1", bufs=3, name=f"m1{wname}{m}{st}"
                    )
                    nc.vector.tensor_mul(m1[:], p_, sin_sb[:, sl])
                    tmp = pool.tile(
                        [P, QW], mdt, tag="ctmp", bufs=3, name=f"tmp{wname}{m}{st}"
                    )
                    nc.vector.tensor_mul(tmp[:], raw[:], cos_sb[:, sl])
                    if wname == "q":
                        nc.vector.tensor_add(dst[m][st][:], tmp[:], m1[:])
                    else:  # per-head padded K tiles
                        for j in range(2):
                            ro_ = j * HD
                            nc.vector.tensor_add(
                                KT[2 * m + j][st][ro_ : ro_ + HD, :],
                                tmp[ro_ : ro_ + HD, :],
                                m1[ro_ : ro_ + HD, :],
                            )

            # Only the m0 halves (heads 0/1) are roped up front: scores for
            # h0/h1 need QT[0] and KT[0..1] only.  The m1 projections+ropes
            # run inside W1 where PE/DVE are otherwise idle.
            rope_group("q", QT, 0, psq0)
            rope_group("k", KT, 0, [psk0[st][:] for st in range(NQT)])

            # ================= pipelined attention stream =================
            def emit_scores_pair(h, pk):
                """Scores + exp (+ causal masks) for one kc-pair of a head."""
                th = h // 2
                ka, kb = 2 * pk, 2 * pk + 1
                k0a, k0b = ka * P, kb * P
                cur = []
                for qt in range(NQT):
                    q0 = qt * QW
                    if k0a >= q0 + QW:
                        continue
                    qsa = max(q0, k0a)
                    vb = k0b < q0 + QW
                    sps = sc_tile(f"sc{h}_{pk}_{qt}")
                    nc.tensor.matmul(
                        sps[:, qsa - q0 : QW],
                        lhsT=KT[h][ka // 4][:, (k0a % QW) : (k0a % QW) + P],
                        rhs=QT[th][qt][:, qsa - q0 :],
                        start=True,
                        stop=True,
                    )
                    w2 = 0
                    if vb:
                        qsb = max(q0, k0b)
                        w2 = QW - (qsb - q0)
                        nc.tensor.matmul(
                            sps[:, QW : QW + w2],
                            lhsT=KT[h][kb // 4][:, (k0b % QW) : (k0b % QW) + P],
                            rhs=QT[th][qt][:, qsb - q0 :],
                            start=True,
                            stop=True,
                        )
                    pt = pool.tile(
                        [P, 2 * QW], mdt, tag="pt", bufs=NPT, name=f"pt{h}_{pk}_{qt}"
                    )
                    off = qsa - q0
                    end = QW + w2 if vb else QW
                    nc.scalar.activation(
                        pt[:, off:end], sps[:, off:end], Exp, scale=SCALE
                    )
                    # causal masks on GPSIMD (idle engine)
                    if k0a >= q0:
                        nc.gpsimd.tensor_mul(
                            pt[:, off : off + P], pt[:, off : off + P], dm_sb[:]
                        )
                    if vb and k0b >= q0:
                        nc.gpsimd.tensor_mul(
                            pt[:, QW : QW + P], pt[:, QW : QW + P], dm_sb[:]
                        )
                    cur.append((qt, pt, qsa, q0, vb, ka, kb))
                return cur

            def emit_pv_entries(h, pv, entries, want_qt3):
                for qt, ppt, qsa, q0, vb, ka, kb in entries:
                    if (qt == 3) != want_qt3:
                        continue
                    nc.tensor.matmul(
                        pv[qt][0 : HD + 1, qsa - q0 :],
                        lhsT=VA[ka][:, h * (HD + 1) : (h + 1) * (HD + 1)],
                        rhs=ppt[:, qsa - q0 : QW],
                        start=(ka == 0),
                        stop=(ka == 4 * qt + 3),
                    )
                    if vb:
                        qsb = max(q0, kb * P)
                        w2 = QW - (qsb - q0)
                        nc.tensor.matmul(
                            pv[qt][0 : HD + 1, qsb - q0 :],
                            lhsT=VA[kb][:, h * (HD + 1) : (h + 1) * (HD + 1)],
                            rhs=ppt[:, QW : QW + w2],
                            start=False,
                            stop=(kb == 4 * qt + 3),
                        )

            def divide_store(h, qt, pv):
                # normalize: 1/d = exp(-ln d) on ACT (one table set covers
                # exp+ln), denom broadcast over 64 partitions via ones-outer
                # on the PE
                th, ro = h // 2, (h % 2) * HD
                dn = pool.tile([1, QW], mdt, tag="dn", bufs=2, name=f"dn{h}{qt}")
                nc.vector.tensor_copy(dn[:], pv[qt][HD : HD + 1, :])
                bct = sc_tile(f"bcp{h}{qt}")
                bc_ps = bct[0:HD, 0:QW]
                nc.tensor.matmul(bc_ps, lhsT=ones1[:], rhs=dn[:], start=True, stop=True)
                bc1 = pool.tile([HD, QW], f32, tag="bc1", bufs=2, name=f"bc1{h}{qt}")
                nc.scalar.activation(bc1[:], bc_ps, Ln)
                bc = pool.tile([HD, QW], mdt, tag="bc", bufs=2, name=f"bc{h}{qt}")
                nc.scalar.activation(bc[:], bc1[:], Exp, scale=-1.0)
                nc.vector.tensor_mul(
                    attnT[th][qt][ro : ro + HD, :], pv[qt][0:HD, :], bc[:]
                )

            def km1_group(d_):
                for st in range(NQT):
                    nc.tensor.matmul(
                        pvk[st][:],
                        lhsT=wk_sb[1, d_][:],
                        rhs=x_sb[d_][:, st * QW : (st + 1) * QW],
                        start=(d_ == 0),
                        stop=(d_ == NDC - 1),
                    )

            def v_group_half(g, half, psv):
                for d_ in range(4 * half, 4 * half + 4):
                    for j in range(4):
                        st = 4 * g + j
                        nc.tensor.matmul(
                            psv[j][:, 0:GD],
                            lhsT=x_sb[d_][:, st * P : (st + 1) * P],
                            rhs=w_sb["v", d_][:],
                            start=(d_ == 0),
                            stop=(d_ == NDC - 1),
                        )

            def v_group_drain(g, psv):
                # DVE, not ACT: ACT is saturated by the exp stream
                for j in range(4):
                    st = 4 * g + j
                    va = VA[st]
                    dst_ap = va[:].rearrange("p (h c) -> p h c", c=HD + 1)[:, :, 0:HD]
                    src_ap = psv[j][:, 0:GD].rearrange("p (h c) -> p h c", c=HD)
                    nc.vector.tensor_copy(dst_ap, src_ap)

            sc = {h: {} for h in range(GH)}

            # ---- W1: scores+exp h0; the m1 projections+ropes are emitted
            # after (= lower priority) and hoisted into the exp gaps by the
            # Tile priority scheduler ----
            for pk in range(8):
                sc[0][pk] = emit_scores_pair(0, pk)
            pvq1 = [pv_tile(j, f"psq1{j}") for j in range(NQT)]
            for d_ in range(NDC):
                qproj_group(1, d_, [t[:] for t in pvq1])
            rope_group("q", QT, 1, [t[:] for t in pvq1])
            pvk = [pv_tile(j, f"psk1{j}") for j in range(NQT)]
            for d_ in range(NDC):
                km1_group(d_)
            rope_group("k", KT, 1, [pvk[st][:] for st in range(NQT)])

            # ---- W2: scores+exp h1; V-proj groups and PV h0 + div h0
            # follow at lower priority ----
            for pk in range(8):
                sc[1][pk] = emit_scores_pair(1, pk)
            for g in range(4):
                psv = [pv_tile(j, f"psv{g}") for j in range(NQT)]
                v_group_half(g, 0, psv)
                v_group_half(g, 1, psv)
                v_group_drain(g, psv)
            pv_h = [pv_tile(j, "pvh0") for j in range(NQT)]
            emit_pv_entries(0, pv_h, sc[0][0], False)
            emit_pv_entries(0, pv_h, sc[0][1], False)
            emit_pv_entries(0, pv_h, sc[0][0], True)
            divide_store(0, 0, pv_h)
            keepalive(pv_h[3][:], 4)
            emit_pv_entries(0, pv_h, sc[0][2], False)
            emit_pv_entries(0, pv_h, sc[0][1], True)
            emit_pv_entries(0, pv_h, sc[0][3], False)
            emit_pv_entries(0, pv_h, sc[0][2], True)
            divide_store(0, 1, pv_h)
            keepalive(pv_h[3][:], 4)
            emit_pv_entries(0, pv_h, sc[0][4], False)
            emit_pv_entries(0, pv_h, sc[0][3], True)
            emit_pv_entries(0, pv_h, sc[0][5], False)
            emit_pv_entries(0, pv_h, sc[0][4], True)
            divide_store(0, 2, pv_h)
            keepalive(pv_h[3][:], 4)
            emit_pv_entries(0, pv_h, sc[0][6], False)
            emit_pv_entries(0, pv_h, sc[0][5], True)
            emit_pv_entries(0, pv_h, sc[0][7], False)
            emit_pv_entries(0, pv_h, sc[0][6], True)
            emit_pv_entries(0, pv_h, sc[0][7], True)
            divide_store(0, 3, pv_h)

            # ---- W3/W4: scores h(n+1) | PV h(n-1), div ----
            def pv_window(h_pv, h_sc):
                if h_sc is not None:
                    for pk in range(8):
                        sc[h_sc][pk] = emit_scores_pair(h_sc, pk)
                pv = [pv_tile(j, f"pvh{h_pv}") for j in range(NQT)]
                for pk in range(8):
                    emit_pv_entries(h_pv, pv, sc[h_pv][pk], False)
                    if pk >= 1:
                        emit_pv_entries(h_pv, pv, sc[h_pv][pk - 1], True)
                    if pk == 1:
                        divide_store(h_pv, 0, pv)
                    elif pk == 3:
                        divide_store(h_pv, 1, pv)
                    elif pk == 5:
                        divide_store(h_pv, 2, pv)
                    keepalive(pv[3][:])
                emit_pv_entries(h_pv, pv, sc[h_pv][7], True)
                divide_store(h_pv, 3, pv)

            pv_window(1, 2)
            pv_window(2, 3)

            # ---- W5: PV h3 + div h3 | out-proj interleaved ----
            def emit_outproj_st(st):
                ob = pool.tile([P, D], mdt, tag="ob", bufs=3, name=f"ob{st}")
                opt = sc_tile(f"op{st}")
                for n in range(2):
                    ops = opt[:, n * QW : (n + 1) * QW]
                    for c in range(2):
                        nc.tensor.matmul(
                            ops,
                            lhsT=attnT[c][st // 4][:, (st % 4) * P : (st % 4 + 1) * P],
                            rhs=wo_sb[c][:, n * QW : (n + 1) * QW],
                            start=(c == 0),
                            stop=(c == 1),
                        )
                    if n == 0:
                        nc.vector.tensor_copy(ob[:, 0:QW], ops)
                    else:
                        nc.scalar.activation(ob[:, QW:D], ops, AF_Copy)
                nc.sync.dma_start(out[st * P : (st + 1) * P, :], ob[:])

            pv = [pv_tile(j, "pvh3") for j in range(NQT)]
            ost = {
                1: [0],
                2: [1, 2],
                3: [3, 4],
                4: [5, 6],
                5: [7, 8],
                6: [9, 10],
                7: [11],
            }
            for pk in range(8):
                emit_pv_entries(3, pv, sc[3][pk], False)
                if pk >= 1:
                    emit_pv_entries(3, pv, sc[3][pk - 1], True)
                if pk == 1:
                    divide_store(3, 0, pv)
                elif pk == 3:
                    divide_store(3, 1, pv)
                elif pk == 5:
                    divide_store(3, 2, pv)
                for st in ost.get(pk, []):
                    emit_outproj_st(st)
            emit_pv_entries(3, pv, sc[3][7], True)
            divide_store(3, 3, pv)
            for st in range(12, 16):
                emit_outproj_st(st)

    if split_waits:
        nsplit = _split_excess_waits(nc)
        if nsplit:
            print(f"[kernel] split {nsplit} excess-wait NOPs")
    _cache[key] = nc
    return nc


def _rope_tables():
    inv = 1.0 / (10000.0 ** (np.arange(0, HD, 2, dtype=np.float32) / HD))  # [32]
    t = np.arange(S, dtype=np.float32)
    freqs = np.outer(inv, t)  # [32, S]
    cosb = np.cos(freqs).astype(np.float32)
    sinb = np.sin(freqs).astype(np.float32)
    cosT = np.concatenate([cosb, cosb], axis=0)  # [64, S]
    sinT = np.concatenate([sinb, sinb], axis=0)
    return np.tile(cosT, (2, 1)), np.tile(sinT, (2, 1))  # [128, S]


def _rot_matrix():
    # R @ q  ==  rotate_half(q) per 64-row head block (sign included)
    R = np.zeros((P, P), dtype=np.float32)
    for b in range(2):
        for j in range(32):
            R[b * 64 + j, b * 64 + j + 32] = -1.0
            R[b * 64 + j + 32, b * 64 + j] = 1.0
    return np.ascontiguousarray(R.T)


def _np_mdt():
    if MM_MODE == "bf16":
        import ml_dtypes

        return ml_dtypes.bfloat16
    return np.float32


def make_in_maps(x, Wq, Wk, Wv, Wo):
    ndt = _np_mdt()
    x = np.ascontiguousarray(np.asarray(x, dtype=np.float32))
    Wq, Wk, Wv, Wo = (np.asarray(w, dtype=np.float32) for w in (Wq, Wk, Wv, Wo))
    cos2, sine = _rope_tables()
    kk = np.arange(P)[:, None]
    qq = np.arange(P)[None, :]
    dmask = (kk <= qq).astype(np.float32)
    in_maps = []
    for c in range(NCORES):
        b, g = c // GH, c % GH
        sl = slice(g * GD, (g + 1) * GD)
        wkT = np.ascontiguousarray(Wk[sl, :].T)
        in_maps.append(
            {
                "xT": np.ascontiguousarray(x[b].T).astype(ndt),
                "wqT": np.ascontiguousarray(Wq[sl, :].T).astype(ndt),
                "wk0T": np.ascontiguousarray(wkT[:, 0:P]).astype(ndt),
                "wk1T": np.ascontiguousarray(wkT[:, P : 2 * P]).astype(ndt),
                "wvT": np.ascontiguousarray(Wv[sl, :].T).astype(ndt),
                "woT": np.ascontiguousarray(Wo[:, sl].T).astype(ndt),
                "cos2": cos2.astype(ndt),
                "sine": sine.astype(ndt),
                "dmask": dmask.astype(ndt),
                "rmatT": _rot_matrix().astype(ndt),
            }
        )
    return in_maps


def run(x, Wq, Wk, Wv, Wo, trace=False):
    from concourse.bass_utils import run_bass_kernel_spmd

    nc = build_nc()
    in_maps = make_in_maps(x, Wq, Wk, Wv, Wo)
    res = run_bass_kernel_spmd(nc, in_maps, list(range(NCORES)), trace=trace)
    partials = [
        np.asarray(res.results[c]["out"], dtype=np.float32) for c in range(NCORES)
    ]
    full = np.zeros((B, S, D), dtype=np.float32)
    for c in range(NCORES):
        full[c // GH] += partials[c]
    return full, res


def kernel(x, Wq, Wk, Wv, Wo):
    full, _ = run(x, Wq, Wk, Wv, Wo, trace=False)
    return full
